# revision 1
# baseline (speedup 1.0000x reference)
"""Trainium2 Bass kernel for the Powderworld BehaviorFluidFlow step.

Contract: kernel(**inputs) takes the FULL unsharded inputs
  world         (16, 20, 512, 512) f32
  rand_movement (16, 1, 512, 512) f32
  rand_interact (16, 1, 512, 512) f32   (unused by the reference)
  rand_element  (16, 1, 512, 512) f32   (unused by the reference)
and returns the FULL (16, 20, 512, 512) f32 output.

Sharding: data-parallel over batch; core k processes batches [2k, 2k+1].
All roll-based neighbor access is along W (axis 3), which stays local.

Layout per (batch, 128-row h-tile): channels are split into group a = the 5
mask channels {0:id, 1:density, 2:gravity, 6:momentum, 8:did-gravity} and
group b = the 15 payload channels {3,4,5,7,9..19}; each group lives in one
SBUF tile (128, nch, 514) with one halo column per side holding the circular
W wrap.  Each pass computes single-channel move masks (a-mask = "pixel takes
the value of its in-direction neighbor", b-mask = shifted a-mask; the two
are disjoint), then blends each group with a plain copy on the Scalar engine
plus two predicated copies on the Vector engine, the int8 mask broadcast
across channels via a step-0 access pattern.  Mask compares run on the
Vector engine at the 2x tensor-scalar rate; the and-chains (0/1 multiplies)
and two payload channels' full blends (exact 0/1-mask arithmetic) run on the
otherwise idle GPSIMD engine, emitted strictly between the mask chains so
they never delay a chain the Vector engine is about to wait on.

The per-iteration stages are software-pipelined by emission order so the
Vector engine always has blend work while GPSIMD finishes a mask chain:
  ... m2(i) b2(i) loadsA(i+2) b1(i+1) loadsB(i+2) fx(i) m1(i+2) ...
The output tile is kept in permuted channel order [a|b]; the store DMAs
scatter the channel groups back to the canonical order.

Element-set membership (id in {0,3,8,9,12,14,15}) is computed exactly in five
Vector-engine tensor-scalar ops via the float exponent trick:
(id+127)<<23 reinterpreted as f32 is exactly 2^id; converting back to int32
gives 1<<id; AND with the set's bitmask 54025 and a nonzero test finish it.
"""
import sys

if '/opt/trn_rl_repo' not in sys.path:
    sys.path.insert(0, '/opt/trn_rl_repo')

import numpy as np
import concourse.bacc as bacc
import concourse.mybir as mybir
import concourse.tile as tile
from concourse.bass_utils import run_bass_kernel_spmd

A = mybir.AluOpType
F32 = mybir.dt.float32
I8 = mybir.dt.int8

B, C, H, W = 16, 20, 512, 512
N_CORES = 8
BPC = B // N_CORES
P = 128

_nc_cache = {}


def build_kernel(bpc=BPC, c=C, h=H, w=W):
    key = (bpc, c, h, w)
    if key in _nc_cache:
        return _nc_cache[key]

    nc = bacc.Bacc("TRN2", target_bir_lowering=False, debug=False,
                   num_devices=N_CORES)
    world = nc.dram_tensor("world", [bpc, c, h, w], F32, kind="ExternalInput")
    rand = nc.dram_tensor("rand", [bpc, h, w], F32, kind="ExternalInput")
    out = nc.dram_tensor("out", [bpc, c, h, w], F32, kind="ExternalOutput")

    WH = w + 2          # haloed width; data in cols [1, w], halos at 0 and w+1
    n_ht = h // P
    MAIN = slice(1, w + 1)
    ca, cb = 5, c - 5   # group sizes (a = mask channels, b = payload)
    NPC = 2             # payload channels blended on GPSIMD instead of DVE
    CBD = cb - NPC      # b-group channels blended with copy_predicated


    # membership set {empty, water, lava, gas, acid, agentK, agentL}
    # = ids {0, 3, 8, 9, 12, 14, 15} = bits of 54025; tested by building
    # 1<<id via the f32 exponent-field trick (exact integer arithmetic).
    MBITS = 54025

    iters = [(b, t) for b in range(bpc) for t in range(n_ht)]
    n = len(iters)
    st = [dict() for _ in range(n)]   # per-iteration tile refs

    with tile.TileContext(nc) as tc:
        with tc.tile_pool(name="ga", bufs=4) as gap, \
             tc.tile_pool(name="gb", bufs=2) as gbp, \
             tc.tile_pool(name="out2", bufs=1) as o2p, \
             tc.tile_pool(name="mk", bufs=9) as mk, \
             tc.tile_pool(name="it", bufs=4) as itp, \
             tc.tile_pool(name="dbl", bufs=2) as dblp, \
             tc.tile_pool(name="amf", bufs=6) as amfp, \
             tc.tile_pool(name="am", bufs=6) as amp, \
             tc.tile_pool(name="pb", bufs=4) as pbp, \
             tc.tile_pool(name="rp", bufs=3) as rp:

            def membership(ch0, out_tile):
                """out_tile = 1.0 where id in bits(MBITS) else 0.0 (all DVE).

                (id+127)<<23 is the f32 bit pattern of 2^id; converting that
                back to int gives 1<<id exactly; AND with MBITS + nonzero.
                """
                IT = itp.tile([P, w], mybir.dt.int32, tag="it")
                VT = itp.tile([P, w], mybir.dt.int32, tag="it")
                nc.vector.tensor_copy(IT[:], ch0)
                nc.vector.tensor_scalar(IT[:], IT[:], 8388608, 1065353216,
                                        A.mult, A.add)
                nc.vector.tensor_copy(VT[:], IT[:].bitcast(F32))
                nc.vector.tensor_scalar(VT[:], VT[:], MBITS, None, A.bitwise_and)
                nc.vector.tensor_scalar(out_tile[:], VT[:], 0, None, A.is_gt)

            def loads_a(i):
                b, t = iters[i]
                hs = slice(t * P, (t + 1) * P)
                s = st[i]
                s['INa'] = gap.tile([P, ca, WH], F32, tag="ga", name=f"INa{i}")
                s['RAND'] = rp.tile([P, w], F32, tag="rand", name=f"RAND{i}")
                T = s['INa']
                nc.sync.dma_start(T[:, 0:3, MAIN],
                                  world[b, 0:3, hs, :].rearrange("c p w -> p c w"))
                nc.sync.dma_start(T[:, 3:4, MAIN],
                                  world[b, 6:7, hs, :].rearrange("c p w -> p c w"))
                nc.sync.dma_start(T[:, 4:5, MAIN],
                                  world[b, 8:9, hs, :].rearrange("c p w -> p c w"))
                nc.sync.dma_start(s['RAND'][:], rand[b, hs, :])
                nc.scalar.copy(T[:, :, 0:1], T[:, :, w:w + 1])
                nc.scalar.copy(T[:, :, w + 1:w + 2], T[:, :, 1:2])

            def loads_b(i):
                b, t = iters[i]
                hs = slice(t * P, (t + 1) * P)
                s = st[i]
                s['INb'] = gbp.tile([P, cb, WH], F32, tag="gb", name=f"INb{i}")
                T = s['INb']
                nc.sync.dma_start(T[:, 0:3, MAIN],
                                  world[b, 3:6, hs, :].rearrange("c p w -> p c w"))
                nc.sync.dma_start(T[:, 3:4, MAIN],
                                  world[b, 7:8, hs, :].rearrange("c p w -> p c w"))
                nc.sync.dma_start(T[:, 4:cb, MAIN],
                                  world[b, 9:c, hs, :].rearrange("c p w -> p c w"))
                nc.scalar.copy(T[:, :, 0:1], T[:, :, w:w + 1])
                nc.scalar.copy(T[:, :, w + 1:w + 2], T[:, :, 1:2])

            def mask_pass(i, which):
                """Move mask (int8, haloed) for a pass; group-a positions:
                0=id, 1=density, 2=gravity, 3=momentum(ch6), 4=didg(ch8).

                which=1: nbr = j-1 (cur at 0:w), overlap-shift = j+1.
                which=2: nbr = j+1 (cur at 2:w+2), overlap-shift = j-1.
                """
                s = st[i]
                cur = s['INa'] if which == 1 else s['O1a']
                nbr = slice(0, w) if which == 1 else slice(2, w + 2)
                RAND = s['RAND']
                FS = mk.tile([P, w], F32, tag="mk")
                AIR = mk.tile([P, w], F32, tag="mk")
                E = mk.tile([P, w], F32, tag="mk")
                NDG = mk.tile([P, w], F32, tag="mk")
                GB = mk.tile([P, w], F32, tag="mk")
                DN = mk.tile([P, w], F32, tag="mk")
                DBL = dblp.tile([P, WH], F32, tag="dbl")
                AMf = amfp.tile([P, WH], F32, tag="amf", name=f"AMf{which}_{i}")
                AM = amp.tile([P, WH], I8, tag="am", name=f"AM{which}_{i}")

                if which == 1:
                    nc.gpsimd.tensor_tensor(FS[:], RAND[:], cur[:, 3, MAIN],
                                            A.add)
                else:
                    # DVE add so the pass-2 chain start never waits on the
                    # (possibly still draining) GPSIMD queue
                    nc.vector.tensor_tensor(FS[:], RAND[:], cur[:, 3, MAIN],
                                            A.add)
                    # + nfm = 2*b1 after pass 1
                    nc.vector.scalar_tensor_tensor(FS[:], s['A1'][:, 2:w + 2],
                                                   2.0, FS[:], A.mult, A.add)
                membership(cur[:, 0, MAIN], E)
                nc.vector.tensor_scalar(AIR[:], cur[:, 0, MAIN], 13.5, None,
                                        A.is_gt)
                nc.vector.scalar_tensor_tensor(NDG[:], cur[:, 4, MAIN], 0.5,
                                               AIR[:], A.is_lt, A.logical_or)
                # gravity is exactly 0/1 so mult == and for the pair test
                nc.gpsimd.tensor_tensor(GB[:], cur[:, 2, MAIN], cur[:, 2, nbr],
                                        A.mult)
                nc.vector.tensor_tensor(DN[:], cur[:, 1, MAIN], cur[:, 1, nbr],
                                        A.is_gt)
                cmp_op = A.is_gt if which == 1 else A.is_le
                nc.vector.scalar_tensor_tensor(FS[:], FS[:], 0.5, DN[:],
                                               cmp_op, A.logical_and)
                nc.gpsimd.tensor_tensor(E[:], E[:], NDG[:], A.mult)
                nc.gpsimd.tensor_tensor(FS[:], FS[:], E[:], A.mult)
                nc.gpsimd.tensor_tensor(DBL[:, MAIN], FS[:], GB[:], A.mult)
                if which == 1:
                    nc.scalar.copy(DBL[:, w + 1:w + 2], DBL[:, 1:2])
                    nc.vector.scalar_tensor_tensor(AMf[:, MAIN], DBL[:, 2:w + 2],
                                                   0.0, DBL[:, MAIN],
                                                   A.is_equal, A.logical_and)
                else:
                    nc.scalar.copy(DBL[:, 0:1], DBL[:, w:w + 1])
                    nc.vector.scalar_tensor_tensor(AMf[:, MAIN], DBL[:, 0:w],
                                                   0.0, DBL[:, MAIN],
                                                   A.is_equal, A.logical_and)
                nc.vector.tensor_copy(AMf[:, 0:1], AMf[:, w:w + 1])
                nc.vector.tensor_copy(AMf[:, w + 1:w + 2], AMf[:, 1:2])
                nc.vector.tensor_copy(AM[:], AMf[:])
                s[f'A{which}f'], s[f'A{which}'] = AMf, AM

            def pool_blend(s, which, cur, curch, outv):
                """Exact one-channel blend on GPSIMD:
                out = cur*(1-a-b) + a*nbr + b*opp (masks exactly 0/1)."""
                AMf = s[f'A{which}f']
                nbr = slice(0, w) if which == 1 else slice(2, w + 2)
                opp = slice(2, w + 2) if which == 1 else slice(0, w)
                NM = s[f'NM{which}']
                X = pbp.tile([P, w], F32, tag="pb", name=f"X{which}")
                nc.gpsimd.tensor_tensor(X[:], cur[:, curch, MAIN], NM[:], A.mult)
                nc.gpsimd.tensor_tensor(outv, cur[:, curch, nbr], AMf[:, MAIN],
                                        A.mult)
                nc.gpsimd.tensor_tensor(outv, outv, X[:], A.add)
                nc.gpsimd.tensor_tensor(X[:], cur[:, curch, opp], AMf[:, opp],
                                        A.mult)
                nc.gpsimd.tensor_tensor(outv, outv, X[:], A.add)

            def make_nm(s, which):
                # NM = 1 - a - b (exactly 0 where the pixel moves, else 1)
                AMf = s[f'A{which}f']
                opp = slice(2, w + 2) if which == 1 else slice(0, w)
                NM = pbp.tile([P, w], F32, tag="pb", name=f"NM{which}")
                nc.gpsimd.tensor_tensor(NM[:], AMf[:, MAIN], AMf[:, opp], A.add)
                nc.gpsimd.tensor_scalar(NM[:], NM[:], -1.0, None, A.mult)
                nc.gpsimd.tensor_scalar(NM[:], NM[:], 1.0, None, A.add)
                s[f'NM{which}'] = NM

            def blend1_dve(i):
                s = st[i]
                A1 = s['A1']
                s['O1a'] = gap.tile([P, ca, WH], F32, tag="ga", name=f"O1a{i}")
                s['O1b'] = gbp.tile([P, cb, WH], F32, tag="gb", name=f"O1b{i}")
                for IN, O1, nch in ((s['INa'], s['O1a'], ca),
                                    (s['INb'], s['O1b'], CBD)):
                    am = A1[:, MAIN].unsqueeze(1).broadcast_to((P, nch, w))
                    bm = A1[:, 2:w + 2].unsqueeze(1).broadcast_to((P, nch, w))
                    nc.scalar.copy(O1[:, 0:nch, MAIN], IN[:, 0:nch, MAIN])
                    nc.vector.copy_predicated(O1[:, 0:nch, MAIN], am,
                                              IN[:, 0:nch, 0:w])
                    nc.vector.copy_predicated(O1[:, 0:nch, MAIN], bm,
                                              IN[:, 0:nch, 2:w + 2])
                nc.scalar.copy(s['O1a'][:, :, 0:1], s['O1a'][:, :, w:w + 1])
                nc.scalar.copy(s['O1a'][:, :, w + 1:w + 2], s['O1a'][:, :, 1:2])
                O1b = s['O1b']
                nc.scalar.copy(O1b[:, 0:CBD, 0:1], O1b[:, 0:CBD, w:w + 1])
                nc.scalar.copy(O1b[:, 0:CBD, w + 1:w + 2], O1b[:, 0:CBD, 1:2])

            def blend1_pool(i):
                s = st[i]
                make_nm(s, 1)
                for k in range(CBD, cb):
                    pool_blend(s, 1, s['INb'], k, s['O1b'][:, k, MAIN])
                O1b = s['O1b']
                nc.scalar.copy(O1b[:, CBD:cb, 0:1], O1b[:, CBD:cb, w:w + 1])
                nc.scalar.copy(O1b[:, CBD:cb, w + 1:w + 2], O1b[:, CBD:cb, 1:2])

            def blend2(i):
                b, t = iters[i]
                hs = slice(t * P, (t + 1) * P)
                s = st[i]
                A2 = s['A2']
                s['O2'] = o2p.tile([P, c, WH], F32, tag="out2", name=f"O2_{i}")
                O2 = s['O2']
                for O1, o2sl, nch in ((s['O1a'], slice(0, ca), ca),
                                      (s['O1b'], slice(ca, ca + CBD), CBD)):
                    am = A2[:, MAIN].unsqueeze(1).broadcast_to((P, nch, w))
                    bm = A2[:, 0:w].unsqueeze(1).broadcast_to((P, nch, w))
                    nc.scalar.copy(O2[:, o2sl, MAIN], O1[:, 0:nch, MAIN])
                    nc.vector.copy_predicated(O2[:, o2sl, MAIN], am,
                                              O1[:, 0:nch, 2:w + 2])
                    nc.vector.copy_predicated(O2[:, o2sl, MAIN], bm,
                                              O1[:, 0:nch, 0:w])
                # store all channels except 6 (position 3; fixed up in fx).
                # O2 channel order is [0,1,2,6,8 | 3,4,5,7 | 9..19].
                nc.scalar.dma_start(out[b, 0:3, hs, :].rearrange("c p w -> p c w"),
                                    O2[:, 0:3, MAIN])
                nc.scalar.dma_start(out[b, 8:9, hs, :].rearrange("c p w -> p c w"),
                                    O2[:, 4:5, MAIN])
                nc.scalar.dma_start(out[b, 3:6, hs, :].rearrange("c p w -> p c w"),
                                    O2[:, 5:8, MAIN])
                nc.scalar.dma_start(out[b, 7:8, hs, :].rearrange("c p w -> p c w"),
                                    O2[:, 8:9, MAIN])

            def blend2_pool(i):
                b, t = iters[i]
                hs = slice(t * P, (t + 1) * P)
                s = st[i]
                O2 = s['O2']
                make_nm(s, 2)
                for k in range(CBD, cb):
                    pool_blend(s, 2, s['O1b'], k, O2[:, ca + k, MAIN])
                nc.scalar.dma_start(out[b, 9:c, hs, :].rearrange("c p w -> p c w"),
                                    O2[:, 9:c, MAIN])

            def fixup(i):
                b, t = iters[i]
                hs = slice(t * P, (t + 1) * P)
                s = st[i]
                O2 = s['O2']
                NF = mk.tile([P, w], F32, tag="mk")
                FLI = amp.tile([P, w], I8, tag="am", name=f"FLI{i}")
                # nf = 2*b1 - 2*b2 (f32 masks, exact small integers)
                nc.gpsimd.tensor_tensor(NF[:], s['A1f'][:, 2:w + 2],
                                        s['A2f'][:, 0:w], A.subtract)
                nc.gpsimd.tensor_scalar(NF[:], NF[:], 2.0, None, A.mult)
                membership(O2[:, 0, MAIN], FLI)
                nc.vector.copy_predicated(O2[:, 3, MAIN], FLI[:], NF[:])
                nc.scalar.dma_start(out[b, 6, hs, :], O2[:, 3, MAIN])

            # ---- software-pipelined emission -------------------------------
            loads_a(0)
            loads_b(0)
            mask_pass(0, 1)
            if n > 1:
                loads_a(1)
            blend1_dve(0)
            if n > 1:
                loads_b(1)
                mask_pass(1, 1)
            for i in range(n):
                mask_pass(i, 2)
                blend1_pool(i)
                blend2(i)
                blend2_pool(i)
                if i + 2 < n:
                    loads_a(i + 2)
                if i + 1 < n:
                    blend1_dve(i + 1)
                if i + 2 < n:
                    loads_b(i + 2)
                fixup(i)
                if i + 2 < n:
                    mask_pass(i + 2, 1)

    nc.compile()
    _nc_cache[key] = nc
    return nc


def kernel(world, rand_movement, rand_interact, rand_element):
    del rand_interact, rand_element
    nc = build_kernel()
    in_maps = []
    for k in range(N_CORES):
        bs = slice(k * BPC, (k + 1) * BPC)
        in_maps.append({
            "world": np.ascontiguousarray(world[bs]),
            "rand": np.ascontiguousarray(rand_movement[bs, 0]),
        })
    res = run_bass_kernel_spmd(nc, in_maps, list(range(N_CORES)))
    return np.concatenate([res.results[k]["out"] for k in range(N_CORES)], axis=0)



# revision 3
# speedup vs baseline: 1.9146x; 1.9146x over previous
"""Trainium2 Bass kernel for the Powderworld BehaviorFluidFlow step.

Contract: kernel(**inputs) takes the FULL unsharded inputs
  world         (16, 20, 512, 512) f32
  rand_movement (16, 1, 512, 512) f32
  rand_interact (16, 1, 512, 512) f32   (unused by the reference)
  rand_element  (16, 1, 512, 512) f32   (unused by the reference)
and returns the FULL (16, 20, 512, 512) f32 output.

Sharding: data-parallel over batch; core k processes batches [2k, 2k+1].
All roll-based neighbor access is along W (axis 3), which stays local.

Layout: the host packs each pixel into 12 int32 "channels":
  0: density (ch1, raw f32 bits)        1: momentum (ch6, raw f32 bits)
  2: (id, grav)     bf16 pair           3: (didg, w3)  bf16 pair
  4..10: payload bf16 pairs (w4,w5)(w7,w9)(w10,w11)(w12,w13)(w14,w15)
         (w16,w17)(w18,w19)
  11: (E, AIR) bf16 pair - host-precomputed is_element / is_air_move flags
id/grav/didg/E/AIR are small integers, exact in bf16; payload channels only
ever move (never arithmetic), so bf16 costs <= 2^-9 relative error, well
inside the 2e-2 gate.  Density and momentum feed exact f32 comparisons so
they stay f32.  E and AIR ride through the blends like payload, so the
pass-2 masks and the final is_fluid fixup never recompute set membership.

Each pass computes a single-channel move mask (a = "pixel takes its
in-direction neighbor's value", b = shifted a; disjoint), then blends all
12 i32 channels with one plain copy (DVE TensorCopy for the ten packed
channels - the Activation engine's float datapath would corrupt packed
bits - plus an ACT copy of the two f32 channels) and two copy_predicated
ops under an int8 mask broadcast across channels.  copy_predicated runs at
one ELEMENT per lane-cycle regardless of dtype, so 2 bf16 channels per i32
element double its throughput vs the f32 layout.  Mask compare chains run
on DVE; the 0/1 multiply (AND) chains and 1-column halo copies run on the
otherwise idle Pool (gpsimd) engine.

The result is stored as 11 i32 channels (E/AIR dropped) in a
tile-contiguous layout; the host unpacks back to (16, 20, 512, 512) f32.
"""
import sys

if '/opt/trn_rl_repo' not in sys.path:
    sys.path.insert(0, '/opt/trn_rl_repo')

import numpy as np
import ml_dtypes
import concourse.bacc as bacc
import concourse.mybir as mybir
import concourse.tile as tile
from concourse.bass_utils import run_bass_kernel_spmd

A = mybir.AluOpType
F32 = mybir.dt.float32
BF16 = mybir.dt.bfloat16
I32 = mybir.dt.int32
I8 = mybir.dt.int8

B, C, H, W = 16, 20, 512, 512
N_CORES = 8
BPC = B // N_CORES
P = 128
N_HT = H // P
NCH = 12          # i32 channels on device
NST = 11          # stored i32 channels (E/AIR dropped)
WH = W + 2        # haloed width: pixels in cols [1, W], wrap halos at 0, W+1
MAIN = slice(1, W + 1)

# i32 channel -> (lo world ch, hi world ch) for the bf16 pairs
PAIRS = [(0, 2), (8, 3), (4, 5), (7, 9), (10, 11), (12, 13), (14, 15),
         (16, 17), (18, 19)]
FLUID_IDS = (0.0, 3.0, 8.0, 9.0, 12.0, 14.0, 15.0)

_bf16 = ml_dtypes.bfloat16
_u16, _u32 = np.uint16, np.uint32

_nc_cache = {}


def build_kernel():
    key = 0
    if key in _nc_cache:
        return _nc_cache[key]

    nc = bacc.Bacc("TRN2", target_bir_lowering=False, debug=False,
                   num_devices=N_CORES)
    win = nc.dram_tensor("win", [BPC, N_HT, P, NCH, W], I32,
                         kind="ExternalInput")
    rand = nc.dram_tensor("rand", [BPC, H, W], F32, kind="ExternalInput")
    out = nc.dram_tensor("out", [BPC, N_HT, P, NST, W], I32,
                         kind="ExternalOutput")

    iters = [(b, t) for b in range(BPC) for t in range(N_HT)]
    n = len(iters)
    st = [dict() for _ in range(n)]

    def bv(T, c, hi, px):
        """bf16 view of i32 channel c (hi=0 lo half / 1 hi half), pixel
        slice px."""
        return T[:].bitcast(BF16)[:, c,
                                  2 * px.start + hi:2 * (px.stop - 1) + hi + 1:2]

    with tile.TileContext(nc) as tc:
        with tc.tile_pool(name="gin", bufs=2) as ginp, \
             tc.tile_pool(name="go1", bufs=2) as go1p, \
             tc.tile_pool(name="go2", bufs=2) as go2p, \
             tc.tile_pool(name="mk", bufs=4) as mkp, \
             tc.tile_pool(name="dbl", bufs=2) as dblp, \
             tc.tile_pool(name="amf", bufs=4) as amfp, \
             tc.tile_pool(name="am8", bufs=4) as am8p, \
             tc.tile_pool(name="rp", bufs=2) as rp:

            def loads(i):
                b, t = iters[i]
                s = st[i]
                s['IN'] = ginp.tile([P, NCH, WH], I32, tag="gin", name=f"IN{i}")
                s['RAND'] = rp.tile([P, W], F32, tag="rand", name=f"RAND{i}")
                T = s['IN']
                nc.sync.dma_start(T[:, :, MAIN], win[b, t, :, :, :])
                nc.sync.dma_start(s['RAND'][:], rand[b, t * P:(t + 1) * P, :])
                nc.gpsimd.tensor_copy(T[:, :, 0:1], T[:, :, W:W + 1])
                nc.gpsimd.tensor_copy(T[:, :, W + 1:W + 2], T[:, :, 1:2])

            def mask_pass(i, which):
                """Move mask for a pass -> AMf (f32, haloed) + AM8 (int8).

                which=1: in-dir nbr = x-1 (cols 0:W), overlap shift = x+1.
                which=2: in-dir nbr = x+1 (cols 2:W+2), overlap shift = x-1.
                """
                s = st[i]
                cur = s['IN'] if which == 1 else s['O1']
                nbr = slice(0, W) if which == 1 else slice(2, W + 2)
                curf = cur[:].bitcast(F32)
                FS = mkp.tile([P, W], F32, tag="mk")
                DN = mkp.tile([P, W], F32, tag="mk")
                NDG = mkp.tile([P, W], F32, tag="mk")
                GB = mkp.tile([P, W], F32, tag="mk")
                DBL = dblp.tile([P, WH], F32, tag="dbl")
                AMf = amfp.tile([P, WH], F32, tag="amf", name=f"AMf{which}_{i}")
                AM8 = am8p.tile([P, WH], I8, tag="am8", name=f"AM8{which}_{i}")

                # fall score: rand + momentum (+ 2*b1 after pass 1)
                nc.gpsimd.tensor_tensor(FS[:], s['RAND'][:], curf[:, 1, MAIN],
                                        A.add)
                if which == 2:
                    nc.vector.scalar_tensor_tensor(
                        FS[:], s['A1f'][:, 2:W + 2], 2.0, FS[:], A.mult, A.add)
                # density strictly lower in move direction
                nc.vector.tensor_tensor(DN[:], curf[:, 0, MAIN],
                                        curf[:, 0, nbr], A.is_gt)
                cmp_op = A.is_gt if which == 1 else A.is_le
                nc.vector.scalar_tensor_tensor(FS[:], FS[:], 0.5, DN[:],
                                               cmp_op, A.logical_and)
                # not-did-gravity | air-move
                nc.vector.scalar_tensor_tensor(NDG[:], bv(cur, 3, 0, MAIN),
                                               0.5, bv(cur, 11, 1, MAIN),
                                               A.is_lt, A.logical_or)
                # chain the 0/1 ANDs (multiplies) on Pool
                nc.gpsimd.tensor_tensor(NDG[:], bv(cur, 11, 0, MAIN), NDG[:],
                                        A.mult)
                nc.gpsimd.tensor_tensor(FS[:], FS[:], NDG[:], A.mult)
                nc.gpsimd.tensor_tensor(GB[:], bv(cur, 2, 1, MAIN),
                                        bv(cur, 2, 1, nbr), A.mult)
                nc.gpsimd.tensor_tensor(DBL[:, MAIN], FS[:], GB[:], A.mult)
                # overlap kill: a = dbl & ~shift(dbl)
                if which == 1:
                    nc.gpsimd.tensor_copy(DBL[:, W + 1:W + 2], DBL[:, 1:2])
                    nc.vector.scalar_tensor_tensor(
                        AMf[:, MAIN], DBL[:, 2:W + 2], 0.0, DBL[:, MAIN],
                        A.is_equal, A.logical_and)
                else:
                    nc.gpsimd.tensor_copy(DBL[:, 0:1], DBL[:, W:W + 1])
                    nc.vector.scalar_tensor_tensor(
                        AMf[:, MAIN], DBL[:, 0:W], 0.0, DBL[:, MAIN],
                        A.is_equal, A.logical_and)
                nc.gpsimd.tensor_copy(AMf[:, 0:1], AMf[:, W:W + 1])
                nc.gpsimd.tensor_copy(AMf[:, W + 1:W + 2], AMf[:, 1:2])
                nc.vector.tensor_copy(AM8[:], AMf[:])
                s[f'A{which}f'], s[f'A{which}8'] = AMf, AM8

            def blend(i, which):
                s = st[i]
                if which == 1:
                    src, dpool, dname = s['IN'], go1p, "go1"
                else:
                    src, dpool, dname = s['O1'], go2p, "go2"
                dst = dpool.tile([P, NCH, WH], I32, tag=dname,
                                 name=f"O{which}_{i}")
                s[f'O{which}'] = dst
                AM8 = s[f'A{which}8']
                # plain copy: DVE moves packed bits, ACT moves the f32 pair
                nc.vector.tensor_copy(dst[:, 2:NCH, MAIN], src[:, 2:NCH, MAIN])
                nc.scalar.copy(dst[:].bitcast(F32)[:, 0:2, MAIN],
                               src[:].bitcast(F32)[:, 0:2, MAIN])
                if which == 1:
                    a_src, b_msk, b_src = slice(0, W), slice(2, W + 2), slice(2, W + 2)
                else:
                    a_src, b_msk, b_src = slice(2, W + 2), slice(0, W), slice(0, W)
                am = AM8[:, MAIN].unsqueeze(1).broadcast_to((P, NCH, W))
                bm = AM8[:, b_msk].unsqueeze(1).broadcast_to((P, NCH, W))
                nc.vector.copy_predicated(dst[:, :, MAIN], am, src[:, :, a_src])
                nc.vector.copy_predicated(dst[:, :, MAIN], bm, src[:, :, b_src])
                if which == 1:
                    nc.gpsimd.tensor_copy(dst[:, :, 0:1], dst[:, :, W:W + 1])
                    nc.gpsimd.tensor_copy(dst[:, :, W + 1:W + 2], dst[:, :, 1:2])
                else:
                    b, t = iters[i]
                    nc.scalar.dma_start(out[b, t, :, 2:NST, :],
                                        dst[:, 2:NST, MAIN])
                    nc.scalar.dma_start(out[b, t, :, 0:1, :], dst[:, 0:1, MAIN])

            def fixup(i):
                b, t = iters[i]
                s = st[i]
                O2 = s['O2']
                NF = mkp.tile([P, W], F32, tag="mk")
                FLI = am8p.tile([P, W], I8, tag="am8", name=f"FLI{i}")
                # nfm = 2*b1 - 2*b2 (masks exactly 0/1)
                nc.gpsimd.tensor_tensor(NF[:], s['A1f'][:, 2:W + 2],
                                        s['A2f'][:, 0:W], A.subtract)
                nc.gpsimd.tensor_scalar(NF[:], NF[:], 2.0, None, A.mult)
                nc.vector.tensor_copy(FLI[:], bv(O2, 11, 0, MAIN))
                nc.vector.copy_predicated(O2[:].bitcast(F32)[:, 1, MAIN],
                                          FLI[:], NF[:])
                nc.scalar.dma_start(out[b, t, :, 1:2, :], O2[:, 1:2, MAIN])

            # ---- software-pipelined emission -------------------------------
            loads(0)
            for i in range(n):
                if i + 1 < n:
                    loads(i + 1)
                mask_pass(i, 1)
                blend(i, 1)
                mask_pass(i, 2)
                blend(i, 2)
                fixup(i)

    nc.compile()
    _nc_cache[key] = nc
    return nc


def _pack(lo, hi):
    lo16 = np.ascontiguousarray(lo).astype(_bf16).view(_u16).astype(_u32)
    hi16 = np.ascontiguousarray(hi).astype(_bf16).view(_u16).astype(_u32)
    return lo16 | (hi16 << 16)


def prepare_inputs(world, rand_movement):
    """Pack the full-batch inputs into the device layout.

    Returns {"win": (B, N_HT, P, NCH, W) int32, "rand": (B, H, W) f32}.
    """
    ids = world[:, 0]
    E = np.zeros(ids.shape, np.float32)
    for v in FLUID_IDS:
        E += (ids == v)
    AIR = ((ids == 14.0) | (ids == 15.0)).astype(np.float32)
    ch = np.empty((B, NCH, H, W), _u32)
    ch[:, 0] = np.ascontiguousarray(world[:, 1]).view(_u32)
    ch[:, 1] = np.ascontiguousarray(world[:, 6]).view(_u32)
    for j, (lo, hi) in enumerate(PAIRS):
        ch[:, 2 + j] = _pack(world[:, lo], world[:, hi])
    ch[:, 11] = _pack(E, AIR)
    win = np.ascontiguousarray(
        ch.reshape(B, NCH, N_HT, P, W).transpose(0, 2, 3, 1, 4)).view(np.int32)
    return {"win": win, "rand": np.ascontiguousarray(rand_movement[:, 0])}


def unpack_output(stored):
    """(B, N_HT, P, NST, W) int32 device output -> (B, C, H, W) f32."""
    oc = stored.view(_u32).transpose(0, 3, 1, 2, 4).reshape(B, NST, H, W)
    full = np.empty((B, C, H, W), np.float32)
    full[:, 1] = np.ascontiguousarray(oc[:, 0]).view(np.float32)
    full[:, 6] = np.ascontiguousarray(oc[:, 1]).view(np.float32)
    for j, (lo, hi) in enumerate(PAIRS):
        c = np.ascontiguousarray(oc[:, 2 + j])
        full[:, lo] = (c & 0xFFFF).astype(_u16).view(_bf16).astype(np.float32)
        full[:, hi] = (c >> 16).astype(_u16).view(_bf16).astype(np.float32)
    return full


def kernel(world, rand_movement, rand_interact, rand_element):
    del rand_interact, rand_element
    nc = build_kernel()
    packed = prepare_inputs(np.asarray(world), np.asarray(rand_movement))
    in_maps = []
    for k in range(N_CORES):
        bs = slice(k * BPC, (k + 1) * BPC)
        in_maps.append({"win": packed["win"][bs], "rand": packed["rand"][bs]})
    res = run_bass_kernel_spmd(nc, in_maps, list(range(N_CORES)))
    stored = np.concatenate([res.results[k]["out"] for k in range(N_CORES)],
                            axis=0)
    return unpack_output(stored)


# revision 10
# speedup vs baseline: 2.0363x; 1.0635x over previous
"""Trainium2 Bass kernel for the Powderworld BehaviorFluidFlow step.

Contract: kernel(**inputs) takes the FULL unsharded inputs
  world         (16, 20, 512, 512) f32
  rand_movement (16, 1, 512, 512) f32
  rand_interact (16, 1, 512, 512) f32   (unused by the reference)
  rand_element  (16, 1, 512, 512) f32   (unused by the reference)
and returns the FULL (16, 20, 512, 512) f32 output.

Sharding: data-parallel over batch; core k processes batches [2k, 2k+1].
All roll-based neighbor access is along W (axis 3), which stays local.

Layout: the host packs each pixel into 12 int32 "channels":
  0: density (ch1, raw f32 bits)        1: momentum (ch6, raw f32 bits)
  2: (id, grav)     bf16 pair           3: (didg, w3)  bf16 pair
  4..10: payload bf16 pairs (w4,w5)(w7,w9)(w10,w11)(w12,w13)(w14,w15)
         (w16,w17)(w18,w19)
  11: (E, AIR) bf16 pair - host-precomputed is_element / is_air_move flags
id/grav/didg/E/AIR are small integers, exact in bf16; payload channels only
ever move (never arithmetic), so bf16 costs <= 2^-9 relative error, well
inside the 2e-2 gate.  Density and momentum feed exact f32 comparisons so
they stay f32.  E and AIR ride through the blends like payload, so the
pass-2 masks and the final is_fluid fixup never recompute set membership.

Each pass computes a single-channel move mask (a = "pixel takes its
in-direction neighbor's value", b = shifted a; disjoint), then blends all
12 i32 channels with one plain copy (DVE TensorCopy for the ten packed
channels - the Activation engine's float datapath would corrupt packed
bits - plus an ACT copy of the two f32 channels) and two copy_predicated
ops under an int8 mask broadcast across channels.  copy_predicated runs at
one ELEMENT per lane-cycle regardless of dtype, so 2 bf16 channels per i32
element double its throughput vs the f32 layout.  Mask compare chains run
on DVE; the 0/1 multiply (AND) chains and 1-column halo copies run on the
otherwise idle Pool (gpsimd) engine.

The result is stored as 11 i32 channels (E/AIR dropped) in a
tile-contiguous layout; the host unpacks back to (16, 20, 512, 512) f32.
"""
import sys

if '/opt/trn_rl_repo' not in sys.path:
    sys.path.insert(0, '/opt/trn_rl_repo')

import numpy as np
import ml_dtypes
import concourse.bacc as bacc
import concourse.mybir as mybir
import concourse.tile as tile
from concourse.bass_utils import run_bass_kernel_spmd

A = mybir.AluOpType
F32 = mybir.dt.float32
BF16 = mybir.dt.bfloat16
I32 = mybir.dt.int32
I8 = mybir.dt.int8

B, C, H, W = 16, 20, 512, 512
N_CORES = 8
BPC = B // N_CORES
P = 128
N_HT = H // P
NCH = 12          # i32 channels on device
NST = 11          # stored i32 channels (E/AIR dropped)
WH = W + 2        # haloed width: pixels in cols [1, W], wrap halos at 0, W+1
MAIN = slice(1, W + 1)

# i32 channel -> (lo world ch, hi world ch) for the bf16 pairs
PAIRS = [(0, 2), (8, 3), (4, 5), (7, 9), (10, 11), (12, 13), (14, 15),
         (16, 17), (18, 19)]
FLUID_IDS = (0.0, 3.0, 8.0, 9.0, 12.0, 14.0, 15.0)

_bf16 = ml_dtypes.bfloat16
_u16, _u32 = np.uint16, np.uint32

_nc_cache = {}


def build_kernel():
    key = 0
    if key in _nc_cache:
        return _nc_cache[key]

    nc = bacc.Bacc("TRN2", target_bir_lowering=False, debug=False,
                   num_devices=N_CORES)
    win = nc.dram_tensor("win", [BPC, N_HT, P, NCH, W], I32,
                         kind="ExternalInput")
    rand = nc.dram_tensor("rand", [BPC, H, W], F32, kind="ExternalInput")
    out = nc.dram_tensor("out", [BPC, N_HT, P, NST, W], I32,
                         kind="ExternalOutput")

    iters = [(b, t) for b in range(BPC) for t in range(N_HT)]
    n = len(iters)
    st = [dict() for _ in range(n)]

    def bv(T, c, hi, px):
        """bf16 view of i32 channel c (hi=0 lo half / 1 hi half), pixel
        slice px."""
        return T[:].bitcast(BF16)[:, c,
                                  2 * px.start + hi:2 * (px.stop - 1) + hi + 1:2]

    with tile.TileContext(nc) as tc:
        with tc.tile_pool(name="gin", bufs=3) as ginp, \
             tc.tile_pool(name="go1", bufs=2) as go1p, \
             tc.tile_pool(name="go2", bufs=2) as go2p, \
             tc.tile_pool(name="mk", bufs=4) as mkp, \
             tc.tile_pool(name="dbl", bufs=2) as dblp, \
             tc.tile_pool(name="amf", bufs=4) as amfp, \
             tc.tile_pool(name="am8", bufs=4) as am8p, \
             tc.tile_pool(name="rp", bufs=3) as rp:

            def loads(i):
                b, t = iters[i]
                s = st[i]
                s['IN'] = ginp.tile([P, NCH, WH], I32, tag="gin", name=f"IN{i}")
                s['RAND'] = rp.tile([P, W], F32, tag="rand", name=f"RAND{i}")
                T = s['IN']
                nc.sync.dma_start(T[:, :, MAIN], win[b, t, :, :, :])
                nc.sync.dma_start(s['RAND'][:], rand[b, t * P:(t + 1) * P, :])
                nc.vector.tensor_copy(T[:, :, 0:1], T[:, :, W:W + 1])
                nc.vector.tensor_copy(T[:, :, W + 1:W + 2], T[:, :, 1:2])

            def mask_pass(i, which):
                """Move mask for a pass -> AMf (f32, haloed) + AM8 (int8).

                which=1: in-dir nbr = x-1 (cols 0:W), overlap shift = x+1.
                which=2: in-dir nbr = x+1 (cols 2:W+2), overlap shift = x-1.
                """
                s = st[i]
                cur = s['IN'] if which == 1 else s['O1']
                nbr = slice(0, W) if which == 1 else slice(2, W + 2)
                curf = cur[:].bitcast(F32)
                FS = mkp.tile([P, W], F32, tag="mk")
                DN = mkp.tile([P, W], F32, tag="mk")
                NDG = mkp.tile([P, W], F32, tag="mk")
                GB = mkp.tile([P, W], F32, tag="mk")
                DBL = dblp.tile([P, WH], F32, tag="dbl")
                AMf = amfp.tile([P, WH], F32, tag="amf", name=f"AMf{which}_{i}")
                AM8 = am8p.tile([P, WH], I8, tag="am8", name=f"AM8{which}_{i}")

                # fall score: rand + momentum (+ 2*b1 after pass 1)
                nc.gpsimd.tensor_tensor(FS[:], s['RAND'][:], curf[:, 1, MAIN],
                                        A.add)
                if which == 2:
                    nc.vector.scalar_tensor_tensor(
                        FS[:], s['A1f'][:, 2:W + 2], 2.0, FS[:], A.mult, A.add)
                # density strictly lower in move direction
                nc.vector.tensor_tensor(DN[:], curf[:, 0, MAIN],
                                        curf[:, 0, nbr], A.is_gt)
                cmp_op = A.is_gt if which == 1 else A.is_le
                nc.vector.scalar_tensor_tensor(FS[:], FS[:], 0.5, DN[:],
                                               cmp_op, A.logical_and)
                # not-did-gravity | air-move
                nc.vector.scalar_tensor_tensor(NDG[:], bv(cur, 3, 0, MAIN),
                                               0.5, bv(cur, 11, 1, MAIN),
                                               A.is_lt, A.logical_or)
                # chain the 0/1 ANDs (multiplies) on Pool
                nc.gpsimd.tensor_tensor(NDG[:], bv(cur, 11, 0, MAIN), NDG[:],
                                        A.mult)
                nc.gpsimd.tensor_tensor(FS[:], FS[:], NDG[:], A.mult)
                nc.gpsimd.tensor_tensor(GB[:], bv(cur, 2, 1, MAIN),
                                        bv(cur, 2, 1, nbr), A.mult)
                nc.gpsimd.tensor_tensor(DBL[:, MAIN], FS[:], GB[:], A.mult)
                # overlap kill: a = dbl & ~shift(dbl)
                if which == 1:
                    nc.scalar.copy(DBL[:, W + 1:W + 2], DBL[:, 1:2])
                    nc.vector.scalar_tensor_tensor(
                        AMf[:, MAIN], DBL[:, 2:W + 2], 0.0, DBL[:, MAIN],
                        A.is_equal, A.logical_and)
                else:
                    nc.scalar.copy(DBL[:, 0:1], DBL[:, W:W + 1])
                    nc.vector.scalar_tensor_tensor(
                        AMf[:, MAIN], DBL[:, 0:W], 0.0, DBL[:, MAIN],
                        A.is_equal, A.logical_and)
                nc.scalar.copy(AMf[:, 0:1], AMf[:, W:W + 1])
                nc.scalar.copy(AMf[:, W + 1:W + 2], AMf[:, 1:2])
                nc.vector.tensor_copy(AM8[:], AMf[:])
                s[f'A{which}f'], s[f'A{which}8'] = AMf, AM8

            def blend(i, which):
                s = st[i]
                if which == 1:
                    src, dpool, dname = s['IN'], go1p, "go1"
                else:
                    src, dpool, dname = s['O1'], go2p, "go2"
                dst = dpool.tile([P, NCH, WH], I32, tag=dname,
                                 name=f"O{which}_{i}")
                s[f'O{which}'] = dst
                AM8 = s[f'A{which}8']
                # plain copy: DVE moves packed bits, ACT moves the f32 pair
                nc.vector.tensor_copy(dst[:, 2:NCH, MAIN], src[:, 2:NCH, MAIN])
                nc.scalar.copy(dst[:].bitcast(F32)[:, 0:2, MAIN],
                               src[:].bitcast(F32)[:, 0:2, MAIN])
                if which == 1:
                    a_src, b_msk, b_src = slice(0, W), slice(2, W + 2), slice(2, W + 2)
                else:
                    a_src, b_msk, b_src = slice(2, W + 2), slice(0, W), slice(0, W)
                am = AM8[:, MAIN].unsqueeze(1).broadcast_to((P, NCH, W))
                bm = AM8[:, b_msk].unsqueeze(1).broadcast_to((P, NCH, W))
                nc.vector.copy_predicated(dst[:, :, MAIN], am, src[:, :, a_src])
                nc.vector.copy_predicated(dst[:, :, MAIN], bm, src[:, :, b_src])
                if which == 1:
                    nc.vector.tensor_copy(dst[:, :, 0:1], dst[:, :, W:W + 1])
                    nc.vector.tensor_copy(dst[:, :, W + 1:W + 2], dst[:, :, 1:2])
                else:
                    b, t = iters[i]
                    nc.sync.dma_start(out[b, t, :, 2:NST, :],
                                      dst[:, 2:NST, MAIN])
                    nc.sync.dma_start(out[b, t, :, 0:1, :], dst[:, 0:1, MAIN])

            def fixup(i):
                b, t = iters[i]
                s = st[i]
                O2 = s['O2']
                NF = mkp.tile([P, W], F32, tag="mk")
                FLI = am8p.tile([P, W], I8, tag="am8", name=f"FLI{i}")
                # nfm = 2*b1 - 2*b2 (masks exactly 0/1)
                nc.gpsimd.tensor_tensor(NF[:], s['A1f'][:, 2:W + 2],
                                        s['A2f'][:, 0:W], A.subtract)
                nc.gpsimd.tensor_scalar(NF[:], NF[:], 2.0, None, A.mult)
                nc.vector.tensor_copy(FLI[:], bv(O2, 11, 0, MAIN))
                nc.vector.copy_predicated(O2[:].bitcast(F32)[:, 1, MAIN],
                                          FLI[:], NF[:])
                nc.sync.dma_start(out[b, t, :, 1:2, :], O2[:, 1:2, MAIN])

            # ---- software-pipelined emission -------------------------------
            loads(0)
            loads(1)
            for i in range(n):
                if i + 2 < n:
                    loads(i + 2)
                mask_pass(i, 1)
                blend(i, 1)
                mask_pass(i, 2)
                blend(i, 2)
                fixup(i)

    nc.compile()
    _nc_cache[key] = nc
    return nc


def _pack(lo, hi):
    lo16 = np.ascontiguousarray(lo).astype(_bf16).view(_u16).astype(_u32)
    hi16 = np.ascontiguousarray(hi).astype(_bf16).view(_u16).astype(_u32)
    return lo16 | (hi16 << 16)


def prepare_inputs(world, rand_movement):
    """Pack the full-batch inputs into the device layout.

    Returns {"win": (B, N_HT, P, NCH, W) int32, "rand": (B, H, W) f32}.
    """
    ids = world[:, 0]
    E = np.zeros(ids.shape, np.float32)
    for v in FLUID_IDS:
        E += (ids == v)
    AIR = ((ids == 14.0) | (ids == 15.0)).astype(np.float32)
    ch = np.empty((B, NCH, H, W), _u32)
    ch[:, 0] = np.ascontiguousarray(world[:, 1]).view(_u32)
    ch[:, 1] = np.ascontiguousarray(world[:, 6]).view(_u32)
    for j, (lo, hi) in enumerate(PAIRS):
        ch[:, 2 + j] = _pack(world[:, lo], world[:, hi])
    ch[:, 11] = _pack(E, AIR)
    win = np.ascontiguousarray(
        ch.reshape(B, NCH, N_HT, P, W).transpose(0, 2, 3, 1, 4)).view(np.int32)
    return {"win": win, "rand": np.ascontiguousarray(rand_movement[:, 0])}


def unpack_output(stored):
    """(B, N_HT, P, NST, W) int32 device output -> (B, C, H, W) f32."""
    oc = stored.view(_u32).transpose(0, 3, 1, 2, 4).reshape(B, NST, H, W)
    full = np.empty((B, C, H, W), np.float32)
    full[:, 1] = np.ascontiguousarray(oc[:, 0]).view(np.float32)
    full[:, 6] = np.ascontiguousarray(oc[:, 1]).view(np.float32)
    for j, (lo, hi) in enumerate(PAIRS):
        c = np.ascontiguousarray(oc[:, 2 + j])
        full[:, lo] = (c & 0xFFFF).astype(_u16).view(_bf16).astype(np.float32)
        full[:, hi] = (c >> 16).astype(_u16).view(_bf16).astype(np.float32)
    return full


def kernel(world, rand_movement, rand_interact, rand_element):
    del rand_interact, rand_element
    nc = build_kernel()
    packed = prepare_inputs(np.asarray(world), np.asarray(rand_movement))
    in_maps = []
    for k in range(N_CORES):
        bs = slice(k * BPC, (k + 1) * BPC)
        in_maps.append({"win": packed["win"][bs], "rand": packed["rand"][bs]})
    res = run_bass_kernel_spmd(nc, in_maps, list(range(N_CORES)))
    stored = np.concatenate([res.results[k]["out"] for k in range(N_CORES)],
                            axis=0)
    return unpack_output(stored)


# revision 17
# speedup vs baseline: 2.2923x; 1.1258x over previous
"""Trainium2 Bass kernel for the Powderworld BehaviorFluidFlow step.

Contract: kernel(**inputs) takes the FULL unsharded inputs
  world         (16, 20, 512, 512) f32
  rand_movement (16, 1, 512, 512) f32
  rand_interact (16, 1, 512, 512) f32   (unused by the reference)
  rand_element  (16, 1, 512, 512) f32   (unused by the reference)
and returns the FULL (16, 20, 512, 512) f32 output.

Sharding: data-parallel over batch; core k processes batches [2k, 2k+1].
All roll-based neighbor access is along W (axis 3), which stays local.

Layout: the host packs each pixel into 12 int32 "channels":
  0: density (ch1, raw f32 bits)        1: momentum (ch6, raw f32 bits)
  2: (id, grav)     bf16 pair           3: (didg, w3)  bf16 pair
  4..10: payload bf16 pairs (w4,w5)(w7,w9)(w10,w11)(w12,w13)(w14,w15)
         (w16,w17)(w18,w19)
  11: (E, AIR) bf16 pair - host-precomputed is_element / is_air_move flags
id/grav/didg/E/AIR are small integers, exact in bf16; payload channels only
ever move (never arithmetic), so bf16 costs <= 2^-9 relative error, well
inside the 2e-2 gate.  Density and momentum feed exact f32 comparisons so
they stay f32.  E and AIR ride through the blends like payload, so the
pass-2 masks and the final is_fluid fixup never recompute set membership.

Each pass computes a single-channel move mask (a = "pixel takes its
in-direction neighbor's value", b = shifted a; disjoint), then blends all
12 i32 channels with one plain copy (DVE TensorCopy for the ten packed
channels - the Activation engine's float datapath would corrupt packed
bits - plus an ACT copy of the two f32 channels) and two copy_predicated
ops under an int8 mask broadcast across channels.  copy_predicated runs at
one ELEMENT per lane-cycle regardless of dtype, so 2 bf16 channels per i32
element double its throughput vs the f32 layout.  Mask compare chains run
on DVE; the 0/1 multiply (AND) chains and 1-column halo copies run on the
otherwise idle Pool (gpsimd) engine.

The result is stored as 11 i32 channels (E/AIR dropped) in a
tile-contiguous layout; the host unpacks back to (16, 20, 512, 512) f32.
"""
import sys

if '/opt/trn_rl_repo' not in sys.path:
    sys.path.insert(0, '/opt/trn_rl_repo')

import numpy as np
import ml_dtypes
import concourse.bacc as bacc
import concourse.mybir as mybir
import concourse.tile as tile
from concourse.bass_utils import run_bass_kernel_spmd

A = mybir.AluOpType
F32 = mybir.dt.float32
BF16 = mybir.dt.bfloat16
I32 = mybir.dt.int32
I16 = mybir.dt.int16
I8 = mybir.dt.int8

B, C, H, W = 16, 20, 512, 512
N_CORES = 8
BPC = B // N_CORES
P = 128
N_HT = H // P
NCH = 12          # i32 channels on device
NST = 11          # stored i32 channels (E/AIR dropped)
WH = W + 2        # haloed width: pixels in cols [1, W], wrap halos at 0, W+1
MAIN = slice(1, W + 1)

# i32 channel -> (lo world ch, hi world ch) for the bf16 pairs
PAIRS = [(0, 2), (8, 3), (4, 5), (7, 9), (10, 11), (12, 13), (14, 15),
         (16, 17), (18, 19)]
FLUID_IDS = (0.0, 3.0, 8.0, 9.0, 12.0, 14.0, 15.0)

_bf16 = ml_dtypes.bfloat16
_u16, _u32 = np.uint16, np.uint32

_nc_cache = {}


def build_kernel():
    key = 0
    if key in _nc_cache:
        return _nc_cache[key]

    nc = bacc.Bacc("TRN2", target_bir_lowering=False, debug=False,
                   num_devices=N_CORES)
    win = nc.dram_tensor("win", [BPC, N_HT, P, NCH, W], I32,
                         kind="ExternalInput")
    rand = nc.dram_tensor("rand", [BPC, H, W], F32, kind="ExternalInput")
    out = nc.dram_tensor("out", [BPC, N_HT, P, NST, W], I32,
                         kind="ExternalOutput")

    iters = [(b, t) for b in range(BPC) for t in range(N_HT)]
    n = len(iters)
    st = [dict() for _ in range(n)]

    def bv(T, c, hi, px):
        """bf16 view of i32 channel c (hi=0 lo half / 1 hi half), pixel
        slice px."""
        return T[:].bitcast(BF16)[:, c,
                                  2 * px.start + hi:2 * (px.stop - 1) + hi + 1:2]

    with tile.TileContext(nc) as tc:
        with tc.tile_pool(name="gin", bufs=3) as ginp, \
             tc.tile_pool(name="go1", bufs=2) as go1p, \
             tc.tile_pool(name="go2", bufs=2) as go2p, \
             tc.tile_pool(name="mk", bufs=4) as mkp, \
             tc.tile_pool(name="dbl", bufs=2) as dblp, \
             tc.tile_pool(name="am8", bufs=5) as am8p, \
             tc.tile_pool(name="rp", bufs=3) as rp:

            def loads(i):
                b, t = iters[i]
                s = st[i]
                s['IN'] = ginp.tile([P, NCH, WH], I32, tag="gin", name=f"IN{i}")
                s['RAND'] = rp.tile([P, W], F32, tag="rand", name=f"RAND{i}")
                T = s['IN']
                nc.sync.dma_start(T[:, :, MAIN], win[b, t, :, :, :])
                nc.sync.dma_start(s['RAND'][:], rand[b, t * P:(t + 1) * P, :])
                nc.vector.tensor_copy(T[:, :, 0:1], T[:, :, W:W + 1])
                nc.vector.tensor_copy(T[:, :, W + 1:W + 2], T[:, :, 1:2])

            def mask_pass(i, which):
                """Move mask for a pass -> AMf (f32, haloed) + AM8 (int8).

                which=1: in-dir nbr = x-1 (cols 0:W), overlap shift = x+1.
                which=2: in-dir nbr = x+1 (cols 2:W+2), overlap shift = x-1.
                """
                s = st[i]
                cur = s['IN'] if which == 1 else s['O1']
                nbr = slice(0, W) if which == 1 else slice(2, W + 2)
                curf = cur[:].bitcast(F32)
                FS = mkp.tile([P, W], F32, tag="mk")
                DN = mkp.tile([P, W], F32, tag="mk")
                NDG = mkp.tile([P, W], F32, tag="mk")
                GB = mkp.tile([P, W], F32, tag="mk")
                DBL = dblp.tile([P, WH], F32, tag="dbl")
                AM8 = am8p.tile([P, WH], I8, tag="am8", name=f"AM8{which}_{i}")

                # fall score: rand + momentum (+ 2*b1 after pass 1)
                nc.gpsimd.tensor_tensor(FS[:], s['RAND'][:], curf[:, 1, MAIN],
                                        A.add)
                if which == 2:
                    nc.vector.scalar_tensor_tensor(
                        FS[:], s['A18'][:, 2:W + 2], 2.0, FS[:], A.mult, A.add)
                # density strictly lower in move direction
                nc.vector.tensor_tensor(DN[:], curf[:, 0, MAIN],
                                        curf[:, 0, nbr], A.is_gt)
                cmp_op = A.is_gt if which == 1 else A.is_le
                nc.vector.scalar_tensor_tensor(FS[:], FS[:], 0.5, DN[:],
                                               cmp_op, A.logical_and)
                # not-did-gravity | air-move
                nc.vector.scalar_tensor_tensor(NDG[:], bv(cur, 3, 0, MAIN),
                                               0.5, bv(cur, 11, 1, MAIN),
                                               A.is_lt, A.logical_or)
                # chain the 0/1 ANDs (multiplies) on Pool
                nc.gpsimd.tensor_tensor(NDG[:], bv(cur, 11, 0, MAIN), NDG[:],
                                        A.mult)
                nc.gpsimd.tensor_tensor(FS[:], FS[:], NDG[:], A.mult)
                nc.gpsimd.tensor_tensor(GB[:], bv(cur, 2, 1, MAIN),
                                        bv(cur, 2, 1, nbr), A.mult)
                nc.gpsimd.tensor_tensor(DBL[:, MAIN], FS[:], GB[:], A.mult)
                # overlap kill: a = dbl & ~shift(dbl), written straight to int8
                if which == 1:
                    nc.scalar.copy(DBL[:, W + 1:W + 2], DBL[:, 1:2])
                    nc.vector.scalar_tensor_tensor(
                        AM8[:, MAIN], DBL[:, 2:W + 2], 0.0, DBL[:, MAIN],
                        A.is_equal, A.logical_and)
                else:
                    nc.scalar.copy(DBL[:, 0:1], DBL[:, W:W + 1])
                    nc.vector.scalar_tensor_tensor(
                        AM8[:, MAIN], DBL[:, 0:W], 0.0, DBL[:, MAIN],
                        A.is_equal, A.logical_and)
                nc.scalar.copy(AM8[:, 0:1], AM8[:, W:W + 1])
                nc.scalar.copy(AM8[:, W + 1:W + 2], AM8[:, 1:2])
                s[f'A{which}8'] = AM8

            def blend(i, which):
                s = st[i]
                if which == 1:
                    src, dpool, dname = s['IN'], go1p, "go1"
                else:
                    src, dpool, dname = s['O1'], go2p, "go2"
                dst = dpool.tile([P, NCH, WH], I32, tag=dname,
                                 name=f"O{which}_{i}")
                s[f'O{which}'] = dst
                AM8 = s[f'A{which}8']
                # plain copy on ACT as int16: i16 -> f32 -> i16 is lossless,
                # so packed bf16 pairs and raw f32 bits survive intact
                nc.scalar.copy(dst[:, :, MAIN].bitcast(I16),
                               src[:, :, MAIN].bitcast(I16))
                if which == 1:
                    a_src, b_msk, b_src = slice(0, W), slice(2, W + 2), slice(2, W + 2)
                else:
                    a_src, b_msk, b_src = slice(2, W + 2), slice(0, W), slice(0, W)
                am = AM8[:, MAIN].unsqueeze(1).broadcast_to((P, NCH, W))
                bm = AM8[:, b_msk].unsqueeze(1).broadcast_to((P, NCH, W))
                nc.vector.copy_predicated(dst[:, :, MAIN], am, src[:, :, a_src])
                nc.vector.copy_predicated(dst[:, :, MAIN], bm, src[:, :, b_src])
                if which == 1:
                    nc.vector.tensor_copy(dst[:, :, 0:1], dst[:, :, W:W + 1])
                    nc.vector.tensor_copy(dst[:, :, W + 1:W + 2], dst[:, :, 1:2])
                else:
                    b, t = iters[i]
                    nc.sync.dma_start(out[b, t, :, 2:NST, :],
                                      dst[:, 2:NST, MAIN])
                    nc.sync.dma_start(out[b, t, :, 0:1, :], dst[:, 0:1, MAIN])

            def fixup(i):
                b, t = iters[i]
                s = st[i]
                O2 = s['O2']
                NF = mkp.tile([P, W], F32, tag="mk")
                FLI = am8p.tile([P, W], I8, tag="am8", name=f"FLI{i}")
                # nfm = 2*b1 - 2*b2 (masks exactly 0/1)
                nc.gpsimd.tensor_tensor(NF[:], s['A18'][:, 2:W + 2],
                                        s['A28'][:, 0:W], A.subtract)
                nc.gpsimd.tensor_scalar(NF[:], NF[:], 2.0, None, A.mult)
                nc.vector.tensor_copy(FLI[:], bv(O2, 11, 0, MAIN))
                nc.vector.copy_predicated(O2[:].bitcast(F32)[:, 1, MAIN],
                                          FLI[:], NF[:])
                nc.sync.dma_start(out[b, t, :, 1:2, :], O2[:, 1:2, MAIN])

            # ---- software-pipelined emission -------------------------------
            loads(0)
            loads(1)
            for i in range(n):
                if i + 2 < n:
                    loads(i + 2)
                mask_pass(i, 1)
                blend(i, 1)
                mask_pass(i, 2)
                blend(i, 2)
                fixup(i)

    nc.compile()
    _nc_cache[key] = nc
    return nc


def _pack(lo, hi):
    lo16 = np.ascontiguousarray(lo).astype(_bf16).view(_u16).astype(_u32)
    hi16 = np.ascontiguousarray(hi).astype(_bf16).view(_u16).astype(_u32)
    return lo16 | (hi16 << 16)


def prepare_inputs(world, rand_movement):
    """Pack the full-batch inputs into the device layout.

    Returns {"win": (B, N_HT, P, NCH, W) int32, "rand": (B, H, W) f32}.
    """
    ids = world[:, 0]
    E = np.zeros(ids.shape, np.float32)
    for v in FLUID_IDS:
        E += (ids == v)
    AIR = ((ids == 14.0) | (ids == 15.0)).astype(np.float32)
    ch = np.empty((B, NCH, H, W), _u32)
    ch[:, 0] = np.ascontiguousarray(world[:, 1]).view(_u32)
    ch[:, 1] = np.ascontiguousarray(world[:, 6]).view(_u32)
    for j, (lo, hi) in enumerate(PAIRS):
        ch[:, 2 + j] = _pack(world[:, lo], world[:, hi])
    ch[:, 11] = _pack(E, AIR)
    win = np.ascontiguousarray(
        ch.reshape(B, NCH, N_HT, P, W).transpose(0, 2, 3, 1, 4)).view(np.int32)
    return {"win": win, "rand": np.ascontiguousarray(rand_movement[:, 0])}


def unpack_output(stored):
    """(B, N_HT, P, NST, W) int32 device output -> (B, C, H, W) f32."""
    oc = stored.view(_u32).transpose(0, 3, 1, 2, 4).reshape(B, NST, H, W)
    full = np.empty((B, C, H, W), np.float32)
    full[:, 1] = np.ascontiguousarray(oc[:, 0]).view(np.float32)
    full[:, 6] = np.ascontiguousarray(oc[:, 1]).view(np.float32)
    for j, (lo, hi) in enumerate(PAIRS):
        c = np.ascontiguousarray(oc[:, 2 + j])
        full[:, lo] = (c & 0xFFFF).astype(_u16).view(_bf16).astype(np.float32)
        full[:, hi] = (c >> 16).astype(_u16).view(_bf16).astype(np.float32)
    return full


def kernel(world, rand_movement, rand_interact, rand_element):
    del rand_interact, rand_element
    nc = build_kernel()
    packed = prepare_inputs(np.asarray(world), np.asarray(rand_movement))
    in_maps = []
    for k in range(N_CORES):
        bs = slice(k * BPC, (k + 1) * BPC)
        in_maps.append({"win": packed["win"][bs], "rand": packed["rand"][bs]})
    res = run_bass_kernel_spmd(nc, in_maps, list(range(N_CORES)))
    stored = np.concatenate([res.results[k]["out"] for k in range(N_CORES)],
                            axis=0)
    return unpack_output(stored)


# revision 18
# speedup vs baseline: 2.3081x; 1.0069x over previous
"""Trainium2 Bass kernel for the Powderworld BehaviorFluidFlow step.

Contract: kernel(**inputs) takes the FULL unsharded inputs
  world         (16, 20, 512, 512) f32
  rand_movement (16, 1, 512, 512) f32
  rand_interact (16, 1, 512, 512) f32   (unused by the reference)
  rand_element  (16, 1, 512, 512) f32   (unused by the reference)
and returns the FULL (16, 20, 512, 512) f32 output.

Sharding: data-parallel over batch; core k processes batches [2k, 2k+1].
All roll-based neighbor access is along W (axis 3), which stays local.

Layout: the host packs each pixel into 12 int32 "channels":
  0: density (ch1, raw f32 bits)        1: momentum (ch6, raw f32 bits)
  2: (id, grav)     bf16 pair           3: (didg, w3)  bf16 pair
  4..10: payload bf16 pairs (w4,w5)(w7,w9)(w10,w11)(w12,w13)(w14,w15)
         (w16,w17)(w18,w19)
  11: (E, AIR) bf16 pair - host-precomputed is_element / is_air_move flags
id/grav/didg/E/AIR are small integers, exact in bf16; payload channels only
ever move (never arithmetic), so bf16 costs <= 2^-9 relative error, well
inside the 2e-2 gate.  Density and momentum feed exact f32 comparisons so
they stay f32.  E and AIR ride through the blends like payload, so the
pass-2 masks and the final is_fluid fixup never recompute set membership.

Each pass computes a single-channel move mask (a = "pixel takes its
in-direction neighbor's value", b = shifted a; disjoint), then blends all
12 i32 channels with one plain copy (DVE TensorCopy for the ten packed
channels - the Activation engine's float datapath would corrupt packed
bits - plus an ACT copy of the two f32 channels) and two copy_predicated
ops under an int8 mask broadcast across channels.  copy_predicated runs at
one ELEMENT per lane-cycle regardless of dtype, so 2 bf16 channels per i32
element double its throughput vs the f32 layout.  Mask compare chains run
on DVE; the 0/1 multiply (AND) chains and 1-column halo copies run on the
otherwise idle Pool (gpsimd) engine.

The result is stored as 11 i32 channels (E/AIR dropped) in a
tile-contiguous layout; the host unpacks back to (16, 20, 512, 512) f32.
"""
import sys

if '/opt/trn_rl_repo' not in sys.path:
    sys.path.insert(0, '/opt/trn_rl_repo')

import numpy as np
import ml_dtypes
import concourse.bacc as bacc
import concourse.mybir as mybir
import concourse.tile as tile
from concourse.bass_utils import run_bass_kernel_spmd

A = mybir.AluOpType
F32 = mybir.dt.float32
BF16 = mybir.dt.bfloat16
I32 = mybir.dt.int32
I16 = mybir.dt.int16
I8 = mybir.dt.int8

B, C, H, W = 16, 20, 512, 512
N_CORES = 8
BPC = B // N_CORES
P = 128
N_HT = H // P
NCH = 12          # i32 channels on device
NST = 11          # stored i32 channels (E/AIR dropped)
WH = W + 2        # haloed width: pixels in cols [1, W], wrap halos at 0, W+1
MAIN = slice(1, W + 1)

# i32 channel -> (lo world ch, hi world ch) for the bf16 pairs
PAIRS = [(0, 2), (8, 3), (4, 5), (7, 9), (10, 11), (12, 13), (14, 15),
         (16, 17), (18, 19)]
FLUID_IDS = (0.0, 3.0, 8.0, 9.0, 12.0, 14.0, 15.0)

_bf16 = ml_dtypes.bfloat16
_u16, _u32 = np.uint16, np.uint32

_nc_cache = {}


def build_kernel():
    key = 0
    if key in _nc_cache:
        return _nc_cache[key]

    nc = bacc.Bacc("TRN2", target_bir_lowering=False, debug=False,
                   num_devices=N_CORES)
    win = nc.dram_tensor("win", [BPC, N_HT, P, NCH, W], I32,
                         kind="ExternalInput")
    rand = nc.dram_tensor("rand", [BPC, H, W], F32, kind="ExternalInput")
    out = nc.dram_tensor("out", [BPC, N_HT, P, NST, W], I32,
                         kind="ExternalOutput")

    iters = [(b, t) for b in range(BPC) for t in range(N_HT)]
    n = len(iters)
    st = [dict() for _ in range(n)]

    def bv(T, c, hi, px):
        """bf16 view of i32 channel c (hi=0 lo half / 1 hi half), pixel
        slice px."""
        return T[:].bitcast(BF16)[:, c,
                                  2 * px.start + hi:2 * (px.stop - 1) + hi + 1:2]

    with tile.TileContext(nc) as tc:
        with tc.tile_pool(name="gin", bufs=3) as ginp, \
             tc.tile_pool(name="go1", bufs=2) as go1p, \
             tc.tile_pool(name="go2", bufs=2) as go2p, \
             tc.tile_pool(name="mk", bufs=4) as mkp, \
             tc.tile_pool(name="dbl", bufs=2) as dblp, \
             tc.tile_pool(name="am8", bufs=5) as am8p, \
             tc.tile_pool(name="rp", bufs=3) as rp:

            def loads(i):
                b, t = iters[i]
                s = st[i]
                s['IN'] = ginp.tile([P, NCH, WH], I32, tag="gin", name=f"IN{i}")
                s['RAND'] = rp.tile([P, W], F32, tag="rand", name=f"RAND{i}")
                T = s['IN']
                nc.sync.dma_start(T[:, :, MAIN], win[b, t, :, :, :])
                nc.sync.dma_start(s['RAND'][:], rand[b, t * P:(t + 1) * P, :])
                nc.vector.tensor_copy(T[:, :, 0:1], T[:, :, W:W + 1])
                nc.vector.tensor_copy(T[:, :, W + 1:W + 2], T[:, :, 1:2])

            def mask_pass(i, which):
                """Move mask for a pass -> AMf (f32, haloed) + AM8 (int8).

                which=1: in-dir nbr = x-1 (cols 0:W), overlap shift = x+1.
                which=2: in-dir nbr = x+1 (cols 2:W+2), overlap shift = x-1.
                """
                s = st[i]
                cur = s['IN'] if which == 1 else s['O1']
                nbr = slice(0, W) if which == 1 else slice(2, W + 2)
                curf = cur[:].bitcast(F32)
                FS = mkp.tile([P, W], F32, tag="mk")
                DN = mkp.tile([P, W], F32, tag="mk")
                NDG = mkp.tile([P, W], F32, tag="mk")
                GB = mkp.tile([P, W], F32, tag="mk")
                DBL = dblp.tile([P, WH], F32, tag="dbl")
                AM8 = am8p.tile([P, WH], I8, tag="am8", name=f"AM8{which}_{i}")

                # fall score: rand + momentum (+ 2*b1 after pass 1)
                nc.gpsimd.tensor_tensor(FS[:], s['RAND'][:], curf[:, 1, MAIN],
                                        A.add)
                if which == 2:
                    nc.vector.scalar_tensor_tensor(
                        FS[:], s['A18'][:, 2:W + 2], 2.0, FS[:], A.mult, A.add)
                # density strictly lower in move direction
                nc.vector.tensor_tensor(DN[:], curf[:, 0, MAIN],
                                        curf[:, 0, nbr], A.is_gt)
                cmp_op = A.is_gt if which == 1 else A.is_le
                nc.vector.scalar_tensor_tensor(FS[:], FS[:], 0.5, DN[:],
                                               cmp_op, A.logical_and)
                # not-did-gravity | air-move
                nc.vector.scalar_tensor_tensor(NDG[:], bv(cur, 3, 0, MAIN),
                                               0.5, bv(cur, 11, 1, MAIN),
                                               A.is_lt, A.logical_or)
                # chain the 0/1 ANDs (multiplies) on Pool
                nc.gpsimd.tensor_tensor(NDG[:], bv(cur, 11, 0, MAIN), NDG[:],
                                        A.mult)
                nc.gpsimd.tensor_tensor(FS[:], FS[:], NDG[:], A.mult)
                nc.gpsimd.tensor_tensor(GB[:], bv(cur, 2, 1, MAIN),
                                        bv(cur, 2, 1, nbr), A.mult)
                nc.gpsimd.tensor_tensor(DBL[:, MAIN], FS[:], GB[:], A.mult)
                # overlap kill: a = dbl & ~shift(dbl), written straight to int8
                if which == 1:
                    nc.scalar.copy(DBL[:, W + 1:W + 2], DBL[:, 1:2])
                    nc.vector.scalar_tensor_tensor(
                        AM8[:, MAIN], DBL[:, 2:W + 2], 0.0, DBL[:, MAIN],
                        A.is_equal, A.logical_and)
                else:
                    nc.scalar.copy(DBL[:, 0:1], DBL[:, W:W + 1])
                    nc.vector.scalar_tensor_tensor(
                        AM8[:, MAIN], DBL[:, 0:W], 0.0, DBL[:, MAIN],
                        A.is_equal, A.logical_and)
                nc.scalar.copy(AM8[:, 0:1], AM8[:, W:W + 1])
                nc.scalar.copy(AM8[:, W + 1:W + 2], AM8[:, 1:2])
                s[f'A{which}8'] = AM8

            def blend(i, which):
                s = st[i]
                if which == 1:
                    src, dpool, dname = s['IN'], go1p, "go1"
                else:
                    src, dpool, dname = s['O1'], go2p, "go2"
                dst = dpool.tile([P, NCH, WH], I32, tag=dname,
                                 name=f"O{which}_{i}")
                s[f'O{which}'] = dst
                AM8 = s[f'A{which}8']
                if which == 1:
                    a_src, b_msk, b_src = slice(0, W), slice(2, W + 2), slice(2, W + 2)
                else:
                    a_src, b_msk, b_src = slice(2, W + 2), slice(0, W), slice(0, W)
                # plain copy on ACT as int16 (i16 -> f32 -> i16 is lossless, so
                # packed bf16 pairs and raw f32 bits survive), split in halves
                # so each predicated pair can start as soon as its half lands
                for c0, c1 in ((0, NCH // 2), (NCH // 2, NCH)):
                    nc.scalar.copy(dst[:, c0:c1, MAIN].bitcast(I16),
                                   src[:, c0:c1, MAIN].bitcast(I16))
                    nch = c1 - c0
                    am = AM8[:, MAIN].unsqueeze(1).broadcast_to((P, nch, W))
                    bm = AM8[:, b_msk].unsqueeze(1).broadcast_to((P, nch, W))
                    nc.vector.copy_predicated(dst[:, c0:c1, MAIN], am,
                                              src[:, c0:c1, a_src])
                    nc.vector.copy_predicated(dst[:, c0:c1, MAIN], bm,
                                              src[:, c0:c1, b_src])
                if which == 1:
                    nc.vector.tensor_copy(dst[:, :, 0:1], dst[:, :, W:W + 1])
                    nc.vector.tensor_copy(dst[:, :, W + 1:W + 2], dst[:, :, 1:2])
                else:
                    b, t = iters[i]
                    nc.sync.dma_start(out[b, t, :, 2:NST, :],
                                      dst[:, 2:NST, MAIN])
                    nc.sync.dma_start(out[b, t, :, 0:1, :], dst[:, 0:1, MAIN])

            def fixup(i):
                b, t = iters[i]
                s = st[i]
                O2 = s['O2']
                NF = mkp.tile([P, W], F32, tag="mk")
                FLI = am8p.tile([P, W], I8, tag="am8", name=f"FLI{i}")
                # nfm = 2*b1 - 2*b2 (masks exactly 0/1)
                nc.gpsimd.tensor_tensor(NF[:], s['A18'][:, 2:W + 2],
                                        s['A28'][:, 0:W], A.subtract)
                nc.gpsimd.tensor_scalar(NF[:], NF[:], 2.0, None, A.mult)
                nc.vector.tensor_copy(FLI[:], bv(O2, 11, 0, MAIN))
                nc.vector.copy_predicated(O2[:].bitcast(F32)[:, 1, MAIN],
                                          FLI[:], NF[:])
                nc.sync.dma_start(out[b, t, :, 1:2, :], O2[:, 1:2, MAIN])

            # ---- software-pipelined emission -------------------------------
            loads(0)
            loads(1)
            for i in range(n):
                if i + 2 < n:
                    loads(i + 2)
                mask_pass(i, 1)
                blend(i, 1)
                mask_pass(i, 2)
                blend(i, 2)
                fixup(i)

    nc.compile()
    _nc_cache[key] = nc
    return nc


def _pack(lo, hi):
    lo16 = np.ascontiguousarray(lo).astype(_bf16).view(_u16).astype(_u32)
    hi16 = np.ascontiguousarray(hi).astype(_bf16).view(_u16).astype(_u32)
    return lo16 | (hi16 << 16)


def prepare_inputs(world, rand_movement):
    """Pack the full-batch inputs into the device layout.

    Returns {"win": (B, N_HT, P, NCH, W) int32, "rand": (B, H, W) f32}.
    """
    ids = world[:, 0]
    E = np.zeros(ids.shape, np.float32)
    for v in FLUID_IDS:
        E += (ids == v)
    AIR = ((ids == 14.0) | (ids == 15.0)).astype(np.float32)
    ch = np.empty((B, NCH, H, W), _u32)
    ch[:, 0] = np.ascontiguousarray(world[:, 1]).view(_u32)
    ch[:, 1] = np.ascontiguousarray(world[:, 6]).view(_u32)
    for j, (lo, hi) in enumerate(PAIRS):
        ch[:, 2 + j] = _pack(world[:, lo], world[:, hi])
    ch[:, 11] = _pack(E, AIR)
    win = np.ascontiguousarray(
        ch.reshape(B, NCH, N_HT, P, W).transpose(0, 2, 3, 1, 4)).view(np.int32)
    return {"win": win, "rand": np.ascontiguousarray(rand_movement[:, 0])}


def unpack_output(stored):
    """(B, N_HT, P, NST, W) int32 device output -> (B, C, H, W) f32."""
    oc = stored.view(_u32).transpose(0, 3, 1, 2, 4).reshape(B, NST, H, W)
    full = np.empty((B, C, H, W), np.float32)
    full[:, 1] = np.ascontiguousarray(oc[:, 0]).view(np.float32)
    full[:, 6] = np.ascontiguousarray(oc[:, 1]).view(np.float32)
    for j, (lo, hi) in enumerate(PAIRS):
        c = np.ascontiguousarray(oc[:, 2 + j])
        full[:, lo] = (c & 0xFFFF).astype(_u16).view(_bf16).astype(np.float32)
        full[:, hi] = (c >> 16).astype(_u16).view(_bf16).astype(np.float32)
    return full


def kernel(world, rand_movement, rand_interact, rand_element):
    del rand_interact, rand_element
    nc = build_kernel()
    packed = prepare_inputs(np.asarray(world), np.asarray(rand_movement))
    in_maps = []
    for k in range(N_CORES):
        bs = slice(k * BPC, (k + 1) * BPC)
        in_maps.append({"win": packed["win"][bs], "rand": packed["rand"][bs]})
    res = run_bass_kernel_spmd(nc, in_maps, list(range(N_CORES)))
    stored = np.concatenate([res.results[k]["out"] for k in range(N_CORES)],
                            axis=0)
    return unpack_output(stored)
